# revision 1
# baseline (speedup 1.0000x reference)
"""Trainium2 Bass kernel for nn_Encoder_17978733101771 (2x ARMAConv + GroupNorm + tanh).

Sharding (8 cores): core c owns node-eighth c (10 windows x 128 slots,
bin-packed by in-degree); ALL 4 ARMA stacks live on every core.  Edges live
with their destination window, sorted by source, padded to a uniform
chunks-per-window (CPW); padded edge slots carry slot=128 so their one-hot
selection row is all-zero.

Algebra: with dis[n] = rsqrt(max(deg,1)) masked, and linearity of the
per-stack transforms,
  t=0: agg = dis_d * ( seg(dis_s * x[src]) @ iw + c )
  t=1: agg = dis_d * ( seg(dis_s * S1[src]) @ w_arma + c )
  c    = A @ ew + s * eb,  A = seg(dis_s * edge_attr)   (shared by convs)
so the edge embeddings never materialize, t=0 gathers 256B rows from a
LOCAL dis*x (or AllGathered dis*h, 0.33MB/rank) table, and only the t=1
inter-iteration state tables (dis*S1, 4 stacks wide) need full AllGathers.

Device pipeline per core: build xb table; A-phase; per conv: c'; per t per
window: dma_gather source rows (4 SWDGE queues), one-hot Sel matmuls into
PSUM (segment sum), apply iw/w_arma post-aggregation, epilogue
dis*seg + c' + x@rw + b; t=0 writes the dis*S1 table (AllGather, first
half triggered mid-loop); t=1 does the local stack-mean, GroupNorm, tanh.
"""
import sys

sys.path.insert(0, "/opt/trn_rl_repo")

import heapq

import numpy as np
import ml_dtypes

# problem constants (hardcoded per contract)
N, E = 10000, 160000
F_IN, E_DIM, MID, OUT = 64, 16, 128, 256
K, T = 4, 2
GROUPS = 16
EPS = 1e-5

P = 128
NW = 10                 # windows per core
NC = 8
WTOT = NC * NW          # 80
NSLOT = NW * P          # 1280 node slots per core
HW_ = 5                 # windows in first table half
HW2 = NW - HW_
HSLAB = HW_ * P + 8
HSLAB2 = HW2 * P + 8
HROWS = HW_ * P
HROWS2 = HW2 * P

_BUILD_CACHE = {}


# ----------------------------------------------------------------------------
# Bass program
# ----------------------------------------------------------------------------
def _build_nc(CPW, has_b1, has_b2):
    import concourse.bacc as bacc
    import concourse.bass as bass
    import concourse.mybir as mybir
    import concourse.tile as tile
    from concourse import library_config

    f32 = mybir.dt.float32
    bf16 = mybir.dt.bfloat16
    i16 = mybir.dt.int16
    AF = mybir.ActivationFunctionType
    OP = mybir.AluOpType
    AX = mybir.AxisListType

    F1 = 4 * MID           # 512
    F2 = 4 * OUT           # 1024
    NTILE = WTOT           # 80 node tiles in table order

    nc = bacc.Bacc("TRN2", num_devices=8, num_swdge_queues=4)

    def din(name, shape, dt=f32):
        return nc.dram_tensor(name, shape, dt, kind="ExternalInput")

    # ---- external inputs (per-core data)
    xT_d = din("xT", [F_IN, NSLOT])
    xfull_d = din("xfull", [NTILE * P, F_IN])
    dfull_d = din("dfull", [P, NTILE])           # dis per table tile
    ea_d = din("ea", [P, NW, CPW, E_DIM], bf16)
    dsrc_d = din("dsrc", [P, NW, CPW])
    slot_d = din("slot", [P, NW, CPW])
    idx_d = din("idx", [P, NW * CPW * 8], i16)   # tB-table rows
    idx0_d = din("idx0", [P, NW * CPW * 8], i16)  # xb/hb-table rows
    dcol_d = din("dcol", [P, NW])
    iota_d = din("iota", [P, P])
    ident_d = din("ident", [P, P])
    cw1_d = din("cw1", [E_DIM + 1, 4 * MID], bf16)
    cw2_d = din("cw2", [E_DIM + 1, 4 * OUT], bf16)
    wiw1_d = din("wiw1", [F_IN, F1], bf16)
    wrw1_d = din("wrw1", [F_IN, T, F1])
    b1_d = din("b1", [1, T * F1])
    wa1_d = din("wa1", [P, 4, MID], bf16)
    wiw2_d = din("wiw2", [MID, F2], bf16)
    wrw2_d = din("wrw2", [MID, T, F2])
    b2_d = din("b2", [1, T * F2])
    wa2_d = din("wa2", [P, 8, OUT], bf16)
    g1_d = din("g1", [P, MID])
    bt1_d = din("bt1", [P, MID])
    g2_d = din("g2", [P, OUT])
    bt2_d = din("bt2", [P, OUT])
    out_d = nc.dram_tensor("out", [NSLOT, OUT], f32, kind="ExternalOutput")

    # ---- internal DRAM
    xb_d = nc.dram_tensor("xb", [NTILE * P, P], bf16)   # local dis*x table
    hbi_d = nc.dram_tensor("hbi", [NSLOT, MID], bf16)
    hb_d = nc.dram_tensor("hb", [NC * NSLOT, MID], bf16, addr_space="Shared")
    tB1i = nc.dram_tensor("tB1i", [HSLAB + HSLAB2, F1], bf16)
    tB1 = nc.dram_tensor("tB1", [8 * (HSLAB + HSLAB2), F1], bf16,
                         addr_space="Shared")
    tB2i = nc.dram_tensor("tB2i", [HSLAB + HSLAB2, F2], bf16)
    tB2 = nc.dram_tensor("tB2", [8 * (HSLAB + HSLAB2), F2], bf16,
                         addr_space="Shared")

    ALL = [[0, 1, 2, 3, 4, 5, 6, 7]]

    nc.gpsimd.load_library(library_config.mlp)

    with tile.TileContext(nc) as tc:
        with (
            tc.tile_pool(name="const", bufs=1) as cp_,
            tc.tile_pool(name="work", bufs=2) as wk,
            tc.tile_pool(name="workg", bufs=4) as wkg,
            tc.tile_pool(name="work1", bufs=1) as wk1,
            tc.tile_pool(name="psum1", bufs=1, space="PSUM") as ps1,
            tc.tile_pool(name="psum2", bufs=2, space="PSUM") as ps,
            tc.tile_pool(name="psum0", bufs=1, space="PSUM") as ps0,
            tc.tile_pool(name="psumt", bufs=1, space="PSUM") as pst,
        ):
            # ---------- constants to SBUF
            def load_const(d, shape, dt=f32):
                t = cp_.tile(shape, dt, tag=f"c_{d.name}")
                nc.sync.dma_start(out=t[:], in_=d[:])
                return t

            xT_t = load_const(xT_d, [F_IN, NSLOT])
            dfull_t = load_const(dfull_d, [P, NTILE])
            dsrc_t = load_const(dsrc_d, [P, NW, CPW])
            slot_t = load_const(slot_d, [P, NW, CPW])
            idx_t = load_const(idx_d, [P, NW * CPW * 8], i16)
            idx0_t = load_const(idx0_d, [P, NW * CPW * 8], i16)
            dcol_t = load_const(dcol_d, [P, NW])
            iota_t = load_const(iota_d, [P, P])
            ident_t = load_const(ident_d, [P, P])
            cw1_t = load_const(cw1_d, [E_DIM + 1, 4 * MID], bf16)
            cw2_t = load_const(cw2_d, [E_DIM + 1, 4 * OUT], bf16)
            wiw1_t = load_const(wiw1_d, [F_IN, F1], bf16)
            wrw1_t = load_const(wrw1_d, [F_IN, T, F1])
            b1_t = load_const(b1_d, [1, T * F1])
            wa1_t = load_const(wa1_d, [P, 4, MID], bf16)
            wiw2_t = load_const(wiw2_d, [MID, F2], bf16)
            wrw2_t = load_const(wrw2_d, [MID, T, F2])
            b2_t = load_const(b2_d, [1, T * F2])
            wa2_t = load_const(wa2_d, [P, 8, OUT], bf16)
            g1_t = load_const(g1_d, [P, MID])
            bt1_t = load_const(bt1_d, [P, MID])
            g2_t = load_const(g2_d, [P, OUT])
            bt2_t = load_const(bt2_d, [P, OUT])

            ones1 = cp_.tile([1, P], f32, tag="ones1")
            nc.vector.memset(ones1[:], 1.0)
            eps_t = cp_.tile([P, 1], f32, tag="eps")
            nc.vector.memset(eps_t[:], EPS)

            # big residents
            AT_t = cp_.tile([32, NSLOT], bf16, tag="AT")    # A'^T rows 0..16
            hT_t = cp_.tile([MID, NSLOT], f32, tag="hT")    # conv2 dense lhsT

            # zero pad rows of the tB table_in buffers
            zpad = cp_.tile([8, F2], bf16, tag="zpad")
            nc.vector.memset(zpad[:], 0)
            for tin, wd in ((tB1i, F1), (tB2i, F2)):
                nc.sync.dma_start(out=tin[HROWS:HSLAB, :],
                                  in_=zpad[:, :wd])
                nc.sync.dma_start(
                    out=tin[HSLAB + HROWS2:HSLAB + HSLAB2, :],
                    in_=zpad[:, :wd])

            # ---------- xb table: dis * x (bf16, 256B rows), local.
            # Only cols [0:64] are written; the rest is garbage that the
            # consumer (agT[0:Fin]) never reads.
            GB = 8
            for ti in range(0, NTILE, GB):
                r0 = ti * P
                xf = wk.tile([P, GB, F_IN], f32, tag="xf")
                nc.sync.dma_start(
                    out=xf[:],
                    in_=xfull_d[r0:r0 + GB * P, :].rearrange(
                        "(g p) f -> p g f", p=P))
                xfb = wk.tile([P, GB, P], bf16, tag="xfb")
                dfl = dfull_t[:, ti:ti + GB]
                dfb = bass.AP(dfl.tensor, dfl.offset,
                              [dfl.ap[0], [1, GB], [0, F_IN]])
                nc.vector.tensor_tensor(out=xfb[:, :, :F_IN], in0=xf[:],
                                        in1=dfb, op=OP.mult)
                nc.sync.dma_start(
                    out=xb_d[r0:r0 + GB * P, :].rearrange(
                        "(g p) f -> p g f", p=P),
                    in_=xfb[:])

            # ---------- helpers
            def sel_gen(w):
                sel = wk.tile([P, CPW, P], bf16, tag="sel")
                sl = slot_t[:, w, :]
                in0 = bass.AP(sl.tensor, sl.offset,
                              [sl.ap[0], [1, CPW], [0, P]])
                io = iota_t[:]
                in1 = bass.AP(io.tensor, io.offset,
                              [io.ap[0], [0, CPW], [1, P]])
                nc.vector.tensor_tensor(out=sel[:], in0=in0, in1=in1,
                                        op=OP.is_equal)
                return sel

            # ---------- A-phase: A' = dis_dst * seg(dis_src * [ea | 1])
            for w in range(NW):
                sel = sel_gen(w)
                eaw = wk.tile([P, CPW, E_DIM], bf16, tag="eaw")
                nc.sync.dma_start(out=eaw[:], in_=ea_d[:, w, :, :])
                eam = wk.tile([P, CPW, E_DIM + 1], bf16, tag="eam")
                dsl = dsrc_t[:, w, :]
                dsb = bass.AP(dsl.tensor, dsl.offset,
                              [dsl.ap[0], [1, CPW], [0, E_DIM]])
                nc.vector.tensor_tensor(out=eam[:, :, :E_DIM], in0=eaw[:],
                                        in1=dsb, op=OP.mult)
                nc.vector.tensor_copy(out=eam[:, :, E_DIM:E_DIM + 1],
                                      in_=dsl[:, :, None])
                pA = ps1.tile([P, 32], f32, tag="pdpt", space="PSUM")
                for cc in range(CPW):
                    nc.tensor.matmul(out=pA[:, :E_DIM + 1],
                                     lhsT=sel[:, cc, :], rhs=eam[:, cc, :],
                                     start=(cc == 0), stop=(cc == CPW - 1))
                aq = wk.tile([P, 32], f32, tag="aq")
                nc.vector.memset(aq[:], 0)
                nc.vector.tensor_scalar_mul(aq[:, :E_DIM + 1],
                                            pA[:, :E_DIM + 1],
                                            dcol_t[:, w:w + 1])
                ptr = pst.tile([32, P], f32, tag="ptr", space="PSUM")
                nc.tensor.transpose(out=ptr[:], in_=aq[:],
                                    identity=ident_t[:])
                nc.vector.tensor_copy(
                    out=AT_t[0:E_DIM + 1, w * P:(w + 1) * P],
                    in_=ptr[0:E_DIM + 1, :])

            # ---------- one conv
            def conv(Fc, FW, Fin, xTsrc, cw_t, wiw_t, wrw_t, b_t, wa_t, nkt,
                     tab0, tBi, tB, g_t, bt_t, final, has_b):
                NMM = FW // 512 if FW >= 512 else 1
                MMW = FW // NMM
                HCW = (CPW + 1) // 2
                for t in range(T):
                    for w in range(NW):
                        sel = sel_gen(w)
                        GW = P if t == 0 else FW      # gathered row width
                        idxs = idx0_t if t == 0 else idx_t
                        tab = tab0 if t == 0 else tB
                        if t == 0:
                            praw = ps0.tile([P, P], f32, tag="praw",
                                            space="PSUM")
                            NMMr = 1
                        else:
                            praw = ps.tile([P, FW], f32, tag="pseg",
                                           space="PSUM")
                            NMMr = NMM
                        MMr = GW // NMMr
                        for hw in range(2):
                            c0 = hw * HCW
                            c1 = min(c0 + HCW, CPW)
                            msg = wkg.tile([P, HCW, GW], bf16, tag="msg")
                            step = (c1 - c0 + 1) // 2
                            qn = 2 * hw
                            for a in range(c0, c1, step):
                                b = min(a + step, c1)
                                nc.gpsimd.dma_gather(
                                    msg[:, a - c0:b - c0, :], tab[:],
                                    idxs[:, (w * CPW + a) * 8:
                                         (w * CPW + b) * 8],
                                    (b - a) * P, (b - a) * P, GW,
                                    queue_num=qn % 4)
                                qn += 1
                            for cc in range(c0, c1):
                                for j in range(NMMr):
                                    nc.tensor.matmul(
                                        out=praw[:, j * MMr:(j + 1) * MMr],
                                        lhsT=sel[:, cc, :],
                                        rhs=msg[:, cc - c0,
                                                j * MMr:(j + 1) * MMr],
                                        start=(cc == 0),
                                        stop=(cc == CPW - 1))
                        # per-stack transform applied post-aggregation
                        if t == 0:
                            pseg = ps.tile([P, FW], f32, tag="pseg",
                                           space="PSUM")
                        else:
                            pseg = praw
                        if t == 0:
                            # pseg = seg(dis*x)^T.T @ iw
                            sr = wk.tile([P, P], f32, tag="sraw0")
                            nc.vector.tensor_copy(out=sr[:], in_=praw[:])
                            ptr = pst.tile([P, P], f32, tag="ptr",
                                          space="PSUM")
                            nc.tensor.transpose(out=ptr[:], in_=sr[:],
                                                identity=ident_t[:])
                            agT = wk.tile([P, P], bf16, tag="agT")
                            nc.vector.tensor_copy(out=agT[:], in_=ptr[:])
                            for j in range(NMM):
                                nc.tensor.matmul(
                                    out=pseg[:, j * MMW:(j + 1) * MMW],
                                    lhsT=agT[0:Fin, :],
                                    rhs=wiw_t[:, j * MMW:(j + 1) * MMW],
                                    start=True, stop=True)
                            p2 = pseg
                        else:
                            # p2 = seg(dis*S1) @ w_arma (per stack)
                            sr = wk.tile([P, FW], f32, tag="sraw")
                            nc.vector.tensor_copy(out=sr[:], in_=praw[:])
                            stt = wk.tile([P, FW // P, P], bf16, tag="stt")
                            for ft in range(FW // P):
                                ptr = pst.tile([P, P], f32, tag="ptr",
                                              space="PSUM")
                                nc.tensor.transpose(
                                    out=ptr[:],
                                    in_=sr[:, ft * P:(ft + 1) * P],
                                    identity=ident_t[:])
                                nc.vector.tensor_copy(out=stt[:, ft, :],
                                                      in_=ptr[:])
                            p2 = ps1.tile([P, FW], f32, tag="pdpt",
                                          space="PSUM")
                            for s in range(4):
                                for kt in range(nkt):
                                    nc.tensor.matmul(
                                        out=p2[:, s * Fc:(s + 1) * Fc],
                                        lhsT=stt[:, s * nkt + kt, :],
                                        rhs=wa_t[:, s * nkt + kt, :],
                                        start=(kt == 0),
                                        stop=(kt == nkt - 1))
                        u = wk.tile([P, FW], f32, tag="u")
                        nc.scalar.activation(out=u[:], in_=p2[:],
                                             func=AF.Copy,
                                             scale=dcol_t[:, w:w + 1])
                        pd = ps1.tile([P, FW], f32, tag="pdpt", space="PSUM")
                        for j in range(NMM):
                            nc.tensor.matmul(
                                out=pd[:, j * MMW:(j + 1) * MMW],
                                lhsT=xTsrc[:, w * P:(w + 1) * P],
                                rhs=wrw_t[:, t, j * MMW:(j + 1) * MMW],
                                start=True, stop=False)
                            if has_b:
                                nc.tensor.matmul(
                                    out=pd[:, j * MMW:(j + 1) * MMW],
                                    lhsT=ones1[:],
                                    rhs=b_t[0:1, t * FW + j * MMW:
                                            t * FW + (j + 1) * MMW],
                                    start=False, stop=False)
                            nc.tensor.matmul(
                                out=pd[:, j * MMW:(j + 1) * MMW],
                                lhsT=AT_t[0:E_DIM + 1,
                                          w * P:(w + 1) * P],
                                rhs=cw_t[:, j * MMW:(j + 1) * MMW],
                                start=False, stop=True)
                        sb_ = wk1.tile([P, FW], f32, tag="sb")
                        nc.vector.tensor_tensor(out=sb_[:], in0=u[:],
                                                in1=pd[:], op=OP.add)
                        if t < T - 1:
                            tb = wk.tile([P, FW], bf16, tag="tb")
                            nc.scalar.activation(out=tb[:], in_=sb_[:],
                                                 func=AF.Copy,
                                                 scale=dcol_t[:, w:w + 1])
                            tr = (w * P if w < HW_
                                  else HSLAB + (w - HW_) * P)
                            nc.sync.dma_start(out=tBi[tr:tr + P, :],
                                              in_=tb[:])
                            if w == HW_ - 1:
                                nc.gpsimd.collective_compute(
                                    "AllGather", OP.bypass,
                                    replica_groups=ALL,
                                    ins=[tBi[0:HSLAB, :]],
                                    outs=[tB[0:8 * HSLAB, :]])
                        else:
                            # local mean over 4 stacks -> GroupNorm -> tanh
                            m = wk1.tile([P, Fc], f32, tag="mean")
                            nc.vector.tensor_tensor(
                                out=m[:], in0=sb_[:, 0:Fc],
                                in1=sb_[:, Fc:2 * Fc], op=OP.add)
                            m1 = wk1.tile([P, Fc], f32, tag="mean1")
                            nc.vector.tensor_tensor(
                                out=m1[:], in0=sb_[:, 2 * Fc:3 * Fc],
                                in1=sb_[:, 3 * Fc:4 * Fc], op=OP.add)
                            m2a = wk1.tile([P, Fc], f32, tag="mean2a")
                            nc.vector.tensor_tensor(out=m2a[:], in0=m[:],
                                                    in1=m1[:], op=OP.add)
                            m2 = wk1.tile([P, Fc], f32, tag="mean2")
                            nc.vector.tensor_scalar_mul(m2[:], m2a[:], 0.25)
                            gsz = Fc // GROUPS
                            mg = m2[:].rearrange("p (g s) -> p g s",
                                                 g=GROUPS)
                            red = wk1.tile([P, GROUPS], f32, tag="red")
                            nc.vector.tensor_reduce(out=red[:], in_=mg,
                                                    axis=AX.X, op=OP.add)
                            sq = wk1.tile([P, Fc], f32, tag="sq")
                            nc.scalar.activation(out=sq[:], in_=m2[:],
                                                 func=AF.Square)
                            red2 = wk1.tile([P, GROUPS], f32, tag="red2")
                            nc.vector.tensor_reduce(
                                out=red2[:],
                                in_=sq[:].rearrange("p (g s) -> p g s",
                                                    g=GROUPS),
                                axis=AX.X, op=OP.add)
                            mu = wk1.tile([P, GROUPS], f32, tag="mu")
                            nc.vector.tensor_scalar_mul(mu[:], red[:],
                                                        1.0 / gsz)
                            ex2 = wk1.tile([P, GROUPS], f32, tag="ex2")
                            nc.vector.tensor_scalar_mul(ex2[:], red2[:],
                                                        1.0 / gsz)
                            mu2 = wk1.tile([P, GROUPS], f32, tag="mu2")
                            nc.vector.tensor_tensor(out=mu2[:], in0=mu[:],
                                                    in1=mu[:], op=OP.mult)
                            var = wk1.tile([P, GROUPS], f32, tag="var")
                            nc.vector.tensor_tensor(out=var[:], in0=ex2[:],
                                                    in1=mu2[:],
                                                    op=OP.subtract)
                            sd = wk1.tile([P, GROUPS], f32, tag="sd")
                            nc.scalar.activation(out=sd[:], in_=var[:],
                                                 func=AF.Sqrt,
                                                 bias=eps_t[:])
                            rstd = wk1.tile([P, GROUPS], f32, tag="rstd")
                            nc.vector.reciprocal(out=rstd[:], in_=sd[:])
                            xc = wk1.tile([P, Fc], f32, tag="xc")
                            mua = mu[:]
                            mub = bass.AP(mua.tensor, mua.offset,
                                          [mua.ap[0], [1, GROUPS],
                                           [0, gsz]])
                            nc.vector.tensor_tensor(
                                out=xc[:].rearrange("p (g s) -> p g s",
                                                    g=GROUPS),
                                in0=mg, in1=mub, op=OP.subtract)
                            xn = wk1.tile([P, Fc], f32, tag="xn")
                            rsa = rstd[:]
                            rsb = bass.AP(rsa.tensor, rsa.offset,
                                          [rsa.ap[0], [1, GROUPS],
                                           [0, gsz]])
                            nc.vector.tensor_tensor(
                                out=xn[:].rearrange("p (g s) -> p g s",
                                                    g=GROUPS),
                                in0=xc[:].rearrange("p (g s) -> p g s",
                                                    g=GROUPS),
                                in1=rsb, op=OP.mult)
                            y1 = wk1.tile([P, Fc], f32, tag="y1")
                            nc.vector.tensor_tensor(out=y1[:], in0=xn[:],
                                                    in1=g_t[:], op=OP.mult)
                            y2 = wk1.tile([P, Fc], f32, tag="y2")
                            nc.vector.tensor_tensor(out=y2[:], in0=y1[:],
                                                    in1=bt_t[:], op=OP.add)
                            h = wk1.tile([P, Fc], f32, tag="h")
                            nc.scalar.activation(out=h[:], in_=y2[:],
                                                 func=AF.Tanh)
                            if final:
                                nc.sync.dma_start(
                                    out=out_d[w * P:(w + 1) * P, :],
                                    in_=h[:])
                            else:
                                hbw = wk.tile([P, MID], bf16, tag="hbw")
                                nc.scalar.activation(
                                    out=hbw[:], in_=h[:], func=AF.Copy,
                                    scale=dcol_t[:, w:w + 1])
                                nc.sync.dma_start(
                                    out=hbi_d[w * P:(w + 1) * P, :],
                                    in_=hbw[:])
                                ptr = pst.tile([P, P], f32, tag="ptr",
                                              space="PSUM")
                                nc.tensor.transpose(out=ptr[:], in_=h[:],
                                                    identity=ident_t[:])
                                nc.vector.tensor_copy(
                                    out=hT_t[:, w * P:(w + 1) * P],
                                    in_=ptr[:])
                    if t < T - 1:
                        nc.gpsimd.collective_compute(
                            "AllGather", OP.bypass, replica_groups=ALL,
                            ins=[tBi[HSLAB:HSLAB + HSLAB2, :]],
                            outs=[tB[8 * HSLAB:
                                     8 * (HSLAB + HSLAB2), :]])

            conv(MID, F1, F_IN, xT_t, cw1_t, wiw1_t, wrw1_t, b1_t,
                 wa1_t, 1, xb_d, tB1i, tB1, g1_t, bt1_t, False, has_b1)
            nc.gpsimd.collective_compute(
                "AllGather", OP.bypass, replica_groups=ALL,
                ins=[hbi_d[:]], outs=[hb_d[:]])
            conv(OUT, F2, MID, hT_t, cw2_t, wiw2_t, wrw2_t, b2_t,
                 wa2_t, 2, hb_d, tB2i, tB2, g2_t, bt2_t, True, has_b2)

    nc.compile()
    return nc


# ----------------------------------------------------------------------------
# host preprocessing + run
# ----------------------------------------------------------------------------
def _pack_idxs(flat):
    """Pack flat gather indices (out position g = chunk*128 + partition)
    into the SWDGE dma_gather SBUF layout [128, nchunk*8] int16."""
    nchunk = len(flat) // P
    a = flat.reshape(nchunk, 8, 16)
    sb = np.transpose(a, (2, 0, 1)).reshape(16, nchunk * 8)
    return np.tile(sb, (8, 1)).astype(np.int16)


def kernel(**inputs):
    x = np.asarray(inputs["x"], np.float32)
    ea = np.asarray(inputs["edge_attr"], np.float32)
    ei = np.asarray(inputs["edge_index"])
    src = ei[:, 0].astype(np.int64)
    dst = ei[:, 1].astype(np.int64)

    deg = np.bincount(dst, minlength=N).astype(np.int64)
    dis = np.where(deg > 0, 1.0 / np.sqrt(np.maximum(deg, 1.0)), 0.0)
    dis = dis.astype(np.float32)

    # ---- bin-pack nodes into windows balancing in-degree
    order = np.argsort(-deg, kind="stable")
    heap = [(0, 0, w) for w in range(WTOT)]
    heapq.heapify(heap)
    win_of = np.empty(N, np.int32)
    slot_of = np.empty(N, np.int32)
    for n in order:
        while True:
            esum, cnt, w = heapq.heappop(heap)
            if cnt < P:
                break
        win_of[n] = w
        slot_of[n] = cnt
        heapq.heappush(heap, (esum + int(deg[n]), cnt + 1, w))
    core_of = win_of // NW
    wl_of = win_of % NW
    lrow = wl_of * P + slot_of              # [0, NSLOT) within core

    # ---- edges grouped by dst window, sorted by src
    ewin = win_of[dst]
    ord_e = np.lexsort((src, ewin))
    wcnt = np.bincount(ewin, minlength=WTOT)
    CPW = int(np.ceil(wcnt.max() / P))
    EPW = CPW * P
    starts = np.zeros(WTOT + 1, np.int64)
    np.cumsum(wcnt, out=starts[1:])

    b1 = np.asarray(inputs["b1"], np.float32)
    b2 = np.asarray(inputs["b2"], np.float32)
    has_b1 = bool(np.any(b1))
    has_b2 = bool(np.any(b2))
    key = (CPW, has_b1, has_b2)
    nc = _BUILD_CACHE.get(key)
    if nc is None:
        nc = _build_nc(CPW, has_b1, has_b2)
        _BUILD_CACHE[key] = nc

    iota = np.tile(np.arange(P, dtype=np.float32)[None, :], (P, 1))
    ident = np.eye(P, dtype=np.float32)

    w1 = np.asarray(inputs["w1"], np.float32)
    w2 = np.asarray(inputs["w2"], np.float32)
    iw1 = np.asarray(inputs["iw1"], np.float32)
    iw2 = np.asarray(inputs["iw2"], np.float32)
    rw1 = np.asarray(inputs["rw1"], np.float32)
    rw2 = np.asarray(inputs["rw2"], np.float32)
    ew1 = np.asarray(inputs["ew1"], np.float32)
    ew2 = np.asarray(inputs["ew2"], np.float32)
    eb1 = np.asarray(inputs["eb1"], np.float32)
    eb2 = np.asarray(inputs["eb2"], np.float32)

    bf = ml_dtypes.bfloat16
    ksall = list(range(K))
    shared = {
        "iota": iota,
        "ident": ident,
        "cw1": np.tile(np.concatenate([ew1, eb1[None, :]], 0),
                       (1, 4)).astype(bf),
        "cw2": np.tile(np.concatenate([ew2, eb2[None, :]], 0),
                       (1, 4)).astype(bf),
        "wiw1": np.concatenate([iw1[k] for k in ksall], 1).astype(bf),
        "wrw1": np.stack(
            [np.concatenate([rw1[t, k] for k in ksall], 1)
             for t in range(T)], 1),
        "b1": np.concatenate(
            [np.concatenate([b1[t, k] for k in ksall])
             for t in range(T)])[None, :],
        "wa1": np.stack([w1[0, k] for k in ksall], 1).astype(bf),
        "wiw2": np.concatenate([iw2[k] for k in ksall], 1).astype(bf),
        "wrw2": np.stack(
            [np.concatenate([rw2[t, k] for k in ksall], 1)
             for t in range(T)], 1),
        "b2": np.concatenate(
            [np.concatenate([b2[t, k] for k in ksall])
             for t in range(T)])[None, :],
        "wa2": np.stack(
            [w2[0, k][kt * P:(kt + 1) * P, :]
             for k in ksall for kt in range(2)], 1).astype(bf),
        "g1": np.tile(np.asarray(inputs["gn1_g"], np.float32)[None, :],
                      (P, 1)),
        "bt1": np.tile(np.asarray(inputs["gn1_b"], np.float32)[None, :],
                       (P, 1)),
        "g2": np.tile(np.asarray(inputs["gn2_g"], np.float32)[None, :],
                      (P, 1)),
        "bt2": np.tile(np.asarray(inputs["gn2_b"], np.float32)[None, :],
                       (P, 1)),
    }

    # xfull / dfull in table order (tile = core*NW + wl)
    xfull = np.zeros((WTOT * P, F_IN), np.float32)
    rows = (core_of * NW + wl_of) * P + slot_of
    xfull[rows, :] = x
    dfull = np.zeros((P, WTOT), np.float32)
    dfull[slot_of, core_of * NW + wl_of] = dis
    shared["xfull"] = xfull
    shared["dfull"] = dfull

    # tB table row: half-major, rank-major within half, 8 pad rows/half
    H_of = (wl_of >= HW_).astype(np.int64)
    row_of = np.where(
        H_of == 0,
        core_of * HSLAB + wl_of * P + slot_of,
        8 * HSLAB + core_of * HSLAB2 + (wl_of - HW_) * P + slot_of)
    # xb/hb row: rank-major (table order), no pads
    row0_of = core_of * NSLOT + lrow
    zero_row = HROWS                        # tB: half 0, rank 0 pad row

    in_maps = []
    for c in range(NC):
        idx_all = np.empty((NW, EPW), np.int64)
        idx0_all = np.zeros((NW, EPW), np.int64)
        slot_all = np.full((NW, EPW), P, np.float32)   # pad slot = 128
        dsrc_all = np.zeros((NW, EPW), np.float32)
        ea_all = np.zeros((NW, EPW, E_DIM), np.float32)
        for wl in range(NW):
            w = c * NW + wl
            es = ord_e[starts[w]:starts[w + 1]]
            ne = len(es)
            idx_all[wl, :] = zero_row
            if ne:
                sr = src[es]
                idx_all[wl, :ne] = row_of[sr]
                idx0_all[wl, :ne] = row0_of[sr]
                slot_all[wl, :ne] = slot_of[dst[es]]
                dsrc_all[wl, :ne] = dis[sr]
                ea_all[wl, :ne, :] = ea[es]

        idx_packed = np.concatenate(
            [_pack_idxs(idx_all[wl]) for wl in range(NW)], axis=1)
        idx0_packed = np.concatenate(
            [_pack_idxs(idx0_all[wl]) for wl in range(NW)], axis=1)

        slot_a = slot_all.reshape(NW, CPW, P).transpose(2, 0, 1).copy()
        dsrc_a = dsrc_all.reshape(NW, CPW, P).transpose(2, 0, 1).copy()
        ea_a = (ea_all.reshape(NW, CPW, P, E_DIM)
                .transpose(2, 0, 1, 3).copy())

        cmask = core_of == c
        Xq = np.zeros((NSLOT, F_IN), np.float32)
        Xq[lrow[cmask]] = x[cmask]
        dcol = np.zeros((P, NW), np.float32)
        dcol[slot_of[cmask], wl_of[cmask]] = dis[cmask]

        in_maps.append(dict(shared,
                            xT=np.ascontiguousarray(Xq.T),
                            ea=ea_a.astype(bf), dsrc=dsrc_a, slot=slot_a,
                            idx=idx_packed, idx0=idx0_packed, dcol=dcol))

    from concourse.bass_utils import run_bass_kernel_spmd
    res = run_bass_kernel_spmd(nc, in_maps, core_ids=list(range(8)))
    kernel._last_results = res

    full = np.zeros((N, OUT), np.float32)
    for c in range(NC):
        r = res.results[c]["out"]
        cmask = core_of == c
        full[cmask] = r[lrow[cmask]]
    return full



# revision 6
# speedup vs baseline: 1.7014x; 1.7014x over previous
"""Trainium2 Bass kernel for nn_Encoder_17978733101771 (2x ARMAConv + GroupNorm + tanh).

Sharding (8 cores): core c owns node-eighth c (10 windows x 128 slots,
bin-packed by in-degree); ALL 4 ARMA stacks live on every core.  Edges live
with their destination window, sorted by source, padded to a uniform
chunks-per-window (CPW); padded edge slots carry slot=128 so their one-hot
selection row is all-zero.

Algebra: with dis[n] = rsqrt(max(deg,1)) masked, norm_e = dis_s*dis_d, and
linearity of all per-stack transforms, define per-node graph quantities
  G  = seg(dis_s * x[src])          A = seg(dis_s * ea)     s = seg(dis_s)
  H  = seg(dis_s^2 * G[src])        B = seg(dis_s^2 * A[src])
  Bs = seg(dis_s^2 * s[src])
Then conv1's T=2 ARMA output per stack k is
  S2 = dis*( (H@iw + B@ew + Bs*eb + G@rw0 + s*b0) @ w  + A@ew + s*eb )
       + x@rw1 + b1
so the t=1 state table (4 stacks wide) never materializes: only narrow
node tables are gathered (256B rows).  Conv2 is identical with
G' = seg(dis_s*h[src]), H' = seg(dis_s^2*G'[src]) and the SAME A,s,B,Bs.
All weight products (iw@w etc.) are fused on the host.

Device pipeline per core, 4 gather phases over a shared half-major table
layout (AllGather halves triggered mid-loop):
  P1: gather x-table + ea -> [G|A|s] per window; write UV = dis^2*[G|A|s]
  P2: gather UV -> [H|B|Bs]^T (transposed seg); fused matmuls -> h;
      write hb = dis*h
  P3: gather hb -> G'; write U2 = dis^2*G'
  P4: gather U2 -> H'^T; fused matmuls -> GroupNorm -> tanh -> out
"""
import sys

sys.path.insert(0, "/opt/trn_rl_repo")

import heapq

import numpy as np
import ml_dtypes

# problem constants (hardcoded per contract)
N, E = 10000, 160000
F_IN, E_DIM, MID, OUT = 64, 16, 128, 256
K, T = 4, 2
GROUPS = 16
EPS = 1e-5

P = 128
NW = 10                 # windows per core
NC = 8
WTOT = NC * NW          # 80
NSLOT = NW * P          # 1280 node slots per core
HW_ = 5                 # windows in first table half
HW2 = NW - HW_
HSLAB = HW_ * P + 8
HSLAB2 = HW2 * P + 8
NSLAB = HSLAB + HSLAB2
HROWS = HW_ * P
F1 = 4 * MID            # 512
F2 = 4 * OUT            # 1024

_BUILD_CACHE = {}


# ----------------------------------------------------------------------------
# Bass program
# ----------------------------------------------------------------------------
def _build_nc(CPW, has_b1, has_b2):
    import concourse.bacc as bacc
    import concourse.bass as bass
    import concourse.mybir as mybir
    import concourse.tile as tile
    from concourse import library_config

    f32 = mybir.dt.float32
    bf16 = mybir.dt.bfloat16
    i16 = mybir.dt.int16
    AF = mybir.ActivationFunctionType
    OP = mybir.AluOpType
    AX = mybir.AxisListType

    nc = bacc.Bacc("TRN2", num_devices=8, num_swdge_queues=4)

    def din(name, shape, dt=f32):
        return nc.dram_tensor(name, shape, dt, kind="ExternalInput")

    # ---- external inputs
    xb_d = din("xb", [8 * NSLAB, P], bf16)       # dis*x table (cols 0:64)
    eam_d = din("eam", [P, NW, CPW, 17], bf16)   # [dis_s*ea | dis_s]
    slot_d = din("slot", [P, NW, CPW])
    idx_d = din("idx", [P, NW * CPW * 8], i16)   # table rows (half-major)
    dcol_d = din("dcol", [P, NW])
    dcol2_d = din("dcol2", [P, NW])
    iota_d = din("iota", [P, P])
    ident_d = din("ident", [P, P])
    xT_d = din("xT", [F_IN, NSLOT])
    wh1_d = din("wh1", [F_IN, F1], bf16)
    wg1_d = din("wg1", [F_IN, F1], bf16)
    wbb1_d = din("wbb1", [17, F1], bf16)
    waa1_d = din("waa1", [17, F1], bf16)
    wrw1_d = din("wrw1", [F_IN, F1])
    b1r_d = din("b1r", [1, F1])
    g1_d = din("g1", [P, MID])
    bt1_d = din("bt1", [P, MID])
    wh2_d = din("wh2", [MID, F2], bf16)
    wg2_d = din("wg2", [MID, F2], bf16)
    wbb2_d = din("wbb2", [17, F2], bf16)
    waa2_d = din("waa2", [17, F2], bf16)
    wrw2_d = din("wrw2", [MID, F2])
    b2r_d = din("b2r", [1, F2])
    g2_d = din("g2", [P, OUT])
    bt2_d = din("bt2", [P, OUT])
    out_d = nc.dram_tensor("out", [NSLOT, OUT], f32, kind="ExternalOutput")

    # ---- internal DRAM (one shared half-major table layout for all)
    uvi_d = nc.dram_tensor("uvi", [NSLAB, P], bf16)
    uv_d = nc.dram_tensor("uv", [8 * NSLAB, P], bf16, addr_space="Shared")
    hbi_d = nc.dram_tensor("hbi", [NSLAB, P], bf16)
    hb_d = nc.dram_tensor("hb", [8 * NSLAB, P], bf16, addr_space="Shared")
    u2i_d = nc.dram_tensor("u2i", [NSLAB, P], bf16)
    u2_d = nc.dram_tensor("u2", [8 * NSLAB, P], bf16, addr_space="Shared")

    ALL = [[0, 1, 2, 3, 4, 5, 6, 7]]
    HCW = (CPW + 1) // 2

    nc.gpsimd.load_library(library_config.mlp)

    with tile.TileContext(nc) as tc:
        with (
            tc.tile_pool(name="const", bufs=1) as cp_,
            tc.tile_pool(name="work", bufs=2) as wk,
            tc.tile_pool(name="workg", bufs=4) as wkg,
            tc.tile_pool(name="work1", bufs=1) as wk1,
            tc.tile_pool(name="psq", bufs=2, space="PSUM") as psq,
            tc.tile_pool(name="psf", bufs=2, space="PSUM") as psf,
            tc.tile_pool(name="psd", bufs=1, space="PSUM") as psd,
        ):
            # ---------- constants to SBUF
            def load_const(d, shape, dt=f32):
                t = cp_.tile(shape, dt, tag=f"c_{d.name}")
                nc.sync.dma_start(out=t[:], in_=d[:])
                return t

            slot_t = load_const(slot_d, [P, NW, CPW])
            idx_t = load_const(idx_d, [P, NW * CPW * 8], i16)
            dcol_t = load_const(dcol_d, [P, NW])
            dcol2_t = load_const(dcol2_d, [P, NW])
            iota_t = load_const(iota_d, [P, P])
            ident_t = load_const(ident_d, [P, P])
            xT_t = load_const(xT_d, [F_IN, NSLOT])
            def load_const17(d, width):
                # 17-row weights live at partitions 64:81 so matmul lhsT
                # slices [64:81] share their base partition
                t = cp_.tile([81, width], bf16, tag=f"c_{d.name}")
                nc.sync.dma_start(out=t[64:81, :], in_=d[:])
                return t

            wh1_t = load_const(wh1_d, [F_IN, F1], bf16)
            wg1_t = load_const(wg1_d, [F_IN, F1], bf16)
            wbb1_t = load_const17(wbb1_d, F1)
            waa1_t = load_const17(waa1_d, F1)
            wrw1_t = load_const(wrw1_d, [F_IN, F1])
            b1r_t = load_const(b1r_d, [1, F1])
            g1_t = load_const(g1_d, [P, MID])
            bt1_t = load_const(bt1_d, [P, MID])
            wh2_t = load_const(wh2_d, [MID, F2], bf16)
            wg2_t = load_const(wg2_d, [MID, F2], bf16)
            wbb2_t = load_const17(wbb2_d, F2)
            waa2_t = load_const17(waa2_d, F2)
            wrw2_t = load_const(wrw2_d, [MID, F2])
            b2r_t = load_const(b2r_d, [1, F2])
            g2_t = load_const(g2_d, [P, OUT])
            bt2_t = load_const(bt2_d, [P, OUT])

            ones1 = cp_.tile([1, P], f32, tag="ones1")
            nc.vector.memset(ones1[:], 1.0)
            eps_t = cp_.tile([P, 1], f32, tag="eps")
            nc.vector.memset(eps_t[:], EPS)

            # resident transposed per-window node quantities
            sbT = cp_.tile([96, NSLOT], bf16, tag="sbT")   # [G|A|s]^T
            bbT = cp_.tile([96, NSLOT], bf16, tag="bbT")   # [..|B|Bs]^T
            g2T = cp_.tile([P, NSLOT], bf16, tag="g2T")    # G'^T
            hT = cp_.tile([MID, NSLOT], f32, tag="hT")     # h^T
            selc = cp_.tile([P, NW, CPW, P], bf16, tag="selc")

            # zero the pad rows of the local table slabs
            zpad = cp_.tile([8, P], bf16, tag="zpad")
            nc.vector.memset(zpad[:], 0)
            for tin in (uvi_d, hbi_d, u2i_d):
                nc.sync.dma_start(out=tin[HROWS:HSLAB, :], in_=zpad[:])
                nc.sync.dma_start(out=tin[HSLAB + HW2 * P:NSLAB, :],
                                  in_=zpad[:])

            def sel_gen(w):
                sl = slot_t[:, w, :]
                in0 = bass.AP(sl.tensor, sl.offset,
                              [sl.ap[0], [1, CPW], [0, P]])
                io = iota_t[:]
                in1 = bass.AP(io.tensor, io.offset,
                              [io.ap[0], [0, CPW], [1, P]])
                nc.vector.tensor_tensor(out=selc[:, w], in0=in0, in1=in1,
                                        op=OP.is_equal)

            def gather_win(w, tab, tag, width):
                """Gather this window's edge-source rows: 2 half tiles."""
                msgs = []
                for hw in range(2):
                    c0 = hw * HCW
                    c1 = min(c0 + HCW, CPW)
                    msg = wkg.tile([P, HCW, P], bf16, tag=tag)
                    step = (c1 - c0 + 1) // 2
                    qn = 2 * hw
                    for a in range(c0, c1, step):
                        b = min(a + step, c1)
                        nc.gpsimd.dma_gather(
                            msg[:, a - c0:b - c0, :], tab[:],
                            idx_t[:, (w * CPW + a) * 8:(w * CPW + b) * 8],
                            (b - a) * P, (b - a) * P, P,
                            queue_num=qn % 4)
                        qn += 1
                    msgs.append((c0, c1, msg))
                return msgs

            def tr_of(w):
                return w * P if w < HW_ else HSLAB + (w - HW_) * P

            # ---------- P1: [G|A|s] per window; UV table = dis^2*[G|A|s]
            for w in range(NW):
                sel_gen(w)
                msgs = gather_win(w, xb_d, "m1", P)
                # overwrite cols 64:81 of the gathered rows with eam
                for c0, c1, msg in msgs:
                    nc.sync.dma_start(out=msg[:, 0:c1 - c0, 64:81],
                                      in_=eam_d[:, w, c0:c1, :])
                praw = psq.tile([P, P], f32, tag="pq", space="PSUM")
                for c0, c1, msg in msgs:
                    for cc in range(c0, c1):
                        nc.tensor.matmul(out=praw[:, 0:81],
                                         lhsT=selc[:, w, cc, :],
                                         rhs=msg[:, cc - c0, 0:81],
                                         start=(cc == 0),
                                         stop=(cc == CPW - 1))
                uvt = wk.tile([P, P], bf16, tag="uvt")
                nc.vector.memset(uvt[:], 0)
                nc.vector.tensor_scalar_mul(uvt[:, 0:81], praw[:, 0:81],
                                            dcol2_t[:, w:w + 1])
                tr = tr_of(w)
                nc.sync.dma_start(out=uvi_d[tr:tr + P, :], in_=uvt[:])
                if w == HW_ - 1:
                    nc.gpsimd.collective_compute(
                        "AllGather", OP.bypass, replica_groups=ALL,
                        ins=[uvi_d[0:HSLAB, :]],
                        outs=[uv_d[0:8 * HSLAB, :]])
                sr = wk.tile([P, 81], f32, tag="sr1")
                nc.vector.tensor_copy(out=sr[:], in_=praw[:, 0:81])
                ptr = psq.tile([P, P], f32, tag="pq", space="PSUM")
                nc.tensor.transpose(out=ptr[0:81, :], in_=sr[:],
                                    identity=ident_t[:])
                nc.vector.tensor_copy(out=sbT[0:81, w * P:(w + 1) * P],
                                      in_=ptr[0:81, :])
            nc.gpsimd.collective_compute(
                "AllGather", OP.bypass, replica_groups=ALL,
                ins=[uvi_d[HSLAB:NSLAB, :]],
                outs=[uv_d[8 * HSLAB:8 * NSLAB, :]])

            # ---------- groupnorm + tanh helper
            def gn_tanh(sb_, Fc, g_t, bt_t):
                m = wk1.tile([P, Fc], f32, tag="mean")
                nc.vector.tensor_tensor(out=m[:], in0=sb_[:, 0:Fc],
                                        in1=sb_[:, Fc:2 * Fc], op=OP.add)
                m1 = wk1.tile([P, Fc], f32, tag="mean1")
                nc.vector.tensor_tensor(out=m1[:], in0=sb_[:, 2 * Fc:3 * Fc],
                                        in1=sb_[:, 3 * Fc:4 * Fc], op=OP.add)
                m2a = wk1.tile([P, Fc], f32, tag="mean2a")
                nc.vector.tensor_tensor(out=m2a[:], in0=m[:], in1=m1[:],
                                        op=OP.add)
                m2 = wk1.tile([P, Fc], f32, tag="mean2")
                nc.vector.tensor_scalar_mul(m2[:], m2a[:], 0.25)
                gsz = Fc // GROUPS
                mg = m2[:].rearrange("p (g s) -> p g s", g=GROUPS)
                red = wk1.tile([P, GROUPS], f32, tag="red")
                nc.vector.tensor_reduce(out=red[:], in_=mg, axis=AX.X,
                                        op=OP.add)
                sq = wk1.tile([P, Fc], f32, tag="sq")
                nc.scalar.activation(out=sq[:], in_=m2[:], func=AF.Square)
                red2 = wk1.tile([P, GROUPS], f32, tag="red2")
                nc.vector.tensor_reduce(
                    out=red2[:],
                    in_=sq[:].rearrange("p (g s) -> p g s", g=GROUPS),
                    axis=AX.X, op=OP.add)
                mu = wk1.tile([P, GROUPS], f32, tag="mu")
                nc.vector.tensor_scalar_mul(mu[:], red[:], 1.0 / gsz)
                ex2 = wk1.tile([P, GROUPS], f32, tag="ex2")
                nc.vector.tensor_scalar_mul(ex2[:], red2[:], 1.0 / gsz)
                mu2 = wk1.tile([P, GROUPS], f32, tag="mu2")
                nc.vector.tensor_tensor(out=mu2[:], in0=mu[:], in1=mu[:],
                                        op=OP.mult)
                var = wk1.tile([P, GROUPS], f32, tag="var")
                nc.vector.tensor_tensor(out=var[:], in0=ex2[:], in1=mu2[:],
                                        op=OP.subtract)
                sd = wk1.tile([P, GROUPS], f32, tag="sd")
                nc.scalar.activation(out=sd[:], in_=var[:], func=AF.Sqrt,
                                     bias=eps_t[:])
                rstd = wk1.tile([P, GROUPS], f32, tag="rstd")
                nc.vector.reciprocal(out=rstd[:], in_=sd[:])
                xc = wk1.tile([P, Fc], f32, tag="xc")
                mua = mu[:]
                mub = bass.AP(mua.tensor, mua.offset,
                              [mua.ap[0], [1, GROUPS], [0, gsz]])
                nc.vector.tensor_tensor(
                    out=xc[:].rearrange("p (g s) -> p g s", g=GROUPS),
                    in0=mg, in1=mub, op=OP.subtract)
                xn = wk1.tile([P, Fc], f32, tag="xn")
                rsa = rstd[:]
                rsb = bass.AP(rsa.tensor, rsa.offset,
                              [rsa.ap[0], [1, GROUPS], [0, gsz]])
                nc.vector.tensor_tensor(
                    out=xn[:].rearrange("p (g s) -> p g s", g=GROUPS),
                    in0=xc[:].rearrange("p (g s) -> p g s", g=GROUPS),
                    in1=rsb, op=OP.mult)
                y1 = wk1.tile([P, Fc], f32, tag="y1")
                nc.vector.tensor_tensor(out=y1[:], in0=xn[:], in1=g_t[:],
                                        op=OP.mult)
                y2 = wk1.tile([P, Fc], f32, tag="y2")
                nc.vector.tensor_tensor(out=y2[:], in0=y1[:], in1=bt_t[:],
                                        op=OP.add)
                h = wk1.tile([P, Fc], f32, tag="h")
                nc.scalar.activation(out=h[:], in_=y2[:], func=AF.Tanh)
                return h

            # ---------- P2: conv1 t=1 (transposed seg + fused matmuls)
            for w in range(NW):
                msgs = gather_win(w, uv_d, "m2", P)
                pT = psq.tile([P, P], f32, tag="pq", space="PSUM")
                for c0, c1, msg in msgs:
                    for cc in range(c0, c1):
                        nc.tensor.matmul(out=pT[0:81, :],
                                         lhsT=msg[:, cc - c0, 0:81],
                                         rhs=selc[:, w, cc, :],
                                         start=(cc == 0),
                                         stop=(cc == CPW - 1))
                bhT = wk.tile([96, P], bf16, tag="bhT")
                nc.vector.tensor_copy(out=bhT[0:81, :], in_=pT[0:81, :])
                nc.vector.tensor_copy(
                    out=bbT[64:81, w * P:(w + 1) * P], in_=pT[64:81, :])
                ws = slice(w * P, (w + 1) * P)
                pf = psf.tile([P, F2], f32, tag="pf", space="PSUM")
                nc.tensor.matmul(out=pf[:, 0:F1], lhsT=bhT[0:64, :],
                                 rhs=wh1_t[:], start=True, stop=False)
                nc.tensor.matmul(out=pf[:, 0:F1], lhsT=bhT[64:81, :],
                                 rhs=wbb1_t[64:81, :], start=False, stop=False)
                nc.tensor.matmul(out=pf[:, 0:F1], lhsT=sbT[0:64, ws],
                                 rhs=wg1_t[:], start=False, stop=False)
                nc.tensor.matmul(out=pf[:, 0:F1], lhsT=sbT[64:81, ws],
                                 rhs=waa1_t[64:81, :], start=False, stop=True)
                u = wk1.tile([P, F1], f32, tag="u")
                nc.scalar.activation(out=u[:], in_=pf[:, 0:F1],
                                     func=AF.Copy,
                                     scale=dcol_t[:, w:w + 1])
                pd = psd.tile([P, F2], f32, tag="pd", space="PSUM")
                nc.tensor.matmul(out=pd[:, 0:F1], lhsT=xT_t[:, ws],
                                 rhs=wrw1_t[:], start=True,
                                 stop=not has_b1)
                if has_b1:
                    nc.tensor.matmul(out=pd[:, 0:F1], lhsT=ones1[:],
                                     rhs=b1r_t[:], start=False, stop=True)
                sb_ = wk1.tile([P, F1], f32, tag="sb")
                nc.vector.tensor_tensor(out=sb_[:], in0=u[:],
                                        in1=pd[:, 0:F1], op=OP.add)
                h = gn_tanh(sb_, MID, g1_t, bt1_t)
                hbw = wk.tile([P, P], bf16, tag="hbw")
                nc.scalar.activation(out=hbw[:], in_=h[:], func=AF.Copy,
                                     scale=dcol_t[:, w:w + 1])
                tr = tr_of(w)
                nc.sync.dma_start(out=hbi_d[tr:tr + P, :], in_=hbw[:])
                if w == HW_ - 1:
                    nc.gpsimd.collective_compute(
                        "AllGather", OP.bypass, replica_groups=ALL,
                        ins=[hbi_d[0:HSLAB, :]],
                        outs=[hb_d[0:8 * HSLAB, :]])
                ptr = psq.tile([P, P], f32, tag="pq", space="PSUM")
                nc.tensor.transpose(out=ptr[:], in_=h[:],
                                    identity=ident_t[:])
                nc.vector.tensor_copy(out=hT[:, ws], in_=ptr[:])
            nc.gpsimd.collective_compute(
                "AllGather", OP.bypass, replica_groups=ALL,
                ins=[hbi_d[HSLAB:NSLAB, :]],
                outs=[hb_d[8 * HSLAB:8 * NSLAB, :]])

            # ---------- P3: conv2 t=0 (G'), U2 table = dis^2*G'
            for w in range(NW):
                msgs = gather_win(w, hb_d, "m3", P)
                praw = psq.tile([P, P], f32, tag="pq", space="PSUM")
                for c0, c1, msg in msgs:
                    for cc in range(c0, c1):
                        nc.tensor.matmul(out=praw[:],
                                         lhsT=selc[:, w, cc, :],
                                         rhs=msg[:, cc - c0, :],
                                         start=(cc == 0),
                                         stop=(cc == CPW - 1))
                u2t = wk.tile([P, P], bf16, tag="u2t")
                nc.vector.tensor_scalar_mul(u2t[:], praw[:],
                                            dcol2_t[:, w:w + 1])
                tr = tr_of(w)
                nc.sync.dma_start(out=u2i_d[tr:tr + P, :], in_=u2t[:])
                if w == HW_ - 1:
                    nc.gpsimd.collective_compute(
                        "AllGather", OP.bypass, replica_groups=ALL,
                        ins=[u2i_d[0:HSLAB, :]],
                        outs=[u2_d[0:8 * HSLAB, :]])
                sr = wk.tile([P, P], f32, tag="sr3")
                nc.vector.tensor_copy(out=sr[:], in_=praw[:])
                ptr = psq.tile([P, P], f32, tag="pq", space="PSUM")
                nc.tensor.transpose(out=ptr[:], in_=sr[:],
                                    identity=ident_t[:])
                nc.vector.tensor_copy(out=g2T[:, w * P:(w + 1) * P],
                                      in_=ptr[:])
            nc.gpsimd.collective_compute(
                "AllGather", OP.bypass, replica_groups=ALL,
                ins=[u2i_d[HSLAB:NSLAB, :]],
                outs=[u2_d[8 * HSLAB:8 * NSLAB, :]])

            # ---------- P4: conv2 t=1
            for w in range(NW):
                msgs = gather_win(w, u2_d, "m4", P)
                pT = psq.tile([P, P], f32, tag="pq", space="PSUM")
                for c0, c1, msg in msgs:
                    for cc in range(c0, c1):
                        nc.tensor.matmul(out=pT[:],
                                         lhsT=msg[:, cc - c0, :],
                                         rhs=selc[:, w, cc, :],
                                         start=(cc == 0),
                                         stop=(cc == CPW - 1))
                hpT = wk.tile([P, P], bf16, tag="hpT")
                nc.vector.tensor_copy(out=hpT[:], in_=pT[:])
                ws = slice(w * P, (w + 1) * P)
                pf = psf.tile([P, F2], f32, tag="pf", space="PSUM")
                for j in range(2):
                    js = slice(j * 512, (j + 1) * 512)
                    nc.tensor.matmul(out=pf[:, js], lhsT=hpT[:],
                                     rhs=wh2_t[:, js], start=True,
                                     stop=False)
                    nc.tensor.matmul(out=pf[:, js], lhsT=g2T[:, ws],
                                     rhs=wg2_t[:, js], start=False,
                                     stop=False)
                    nc.tensor.matmul(out=pf[:, js], lhsT=bbT[64:81, ws],
                                     rhs=wbb2_t[64:81, js], start=False,
                                     stop=False)
                    nc.tensor.matmul(out=pf[:, js], lhsT=sbT[64:81, ws],
                                     rhs=waa2_t[64:81, js], start=False,
                                     stop=True)
                u = wk1.tile([P, F2], f32, tag="u2")
                nc.scalar.activation(out=u[:], in_=pf[:], func=AF.Copy,
                                     scale=dcol_t[:, w:w + 1])
                pd = psd.tile([P, F2], f32, tag="pd", space="PSUM")
                for j in range(2):
                    js = slice(j * 512, (j + 1) * 512)
                    nc.tensor.matmul(out=pd[:, js], lhsT=hT[:, ws],
                                     rhs=wrw2_t[:, js], start=True,
                                     stop=not has_b2)
                    if has_b2:
                        nc.tensor.matmul(out=pd[:, js], lhsT=ones1[:],
                                         rhs=b2r_t[:, js], start=False,
                                         stop=True)
                sb_ = wk1.tile([P, F2], f32, tag="sb2")
                nc.vector.tensor_tensor(out=sb_[:], in0=u[:], in1=pd[:],
                                        op=OP.add)
                h = gn_tanh(sb_, OUT, g2_t, bt2_t)
                nc.sync.dma_start(out=out_d[w * P:(w + 1) * P, :],
                                  in_=h[:])

    nc.compile()
    return nc


# ----------------------------------------------------------------------------
# host preprocessing + run
# ----------------------------------------------------------------------------
def _pack_idxs(flat):
    """Pack flat gather indices (out position g = chunk*128 + partition)
    into the SWDGE dma_gather SBUF layout [128, nchunk*8] int16."""
    nchunk = len(flat) // P
    a = flat.reshape(nchunk, 8, 16)
    sb = np.transpose(a, (2, 0, 1)).reshape(16, nchunk * 8)
    return np.tile(sb, (8, 1)).astype(np.int16)


def kernel(**inputs):
    x = np.asarray(inputs["x"], np.float32)
    ea = np.asarray(inputs["edge_attr"], np.float32)
    ei = np.asarray(inputs["edge_index"])
    src = ei[:, 0].astype(np.int64)
    dst = ei[:, 1].astype(np.int64)

    deg = np.bincount(dst, minlength=N).astype(np.int64)
    dis = np.where(deg > 0, 1.0 / np.sqrt(np.maximum(deg, 1.0)), 0.0)
    dis = dis.astype(np.float32)

    # ---- bin-pack nodes into windows balancing in-degree
    order = np.argsort(-deg, kind="stable")
    heap = [(0, 0, w) for w in range(WTOT)]
    heapq.heapify(heap)
    win_of = np.empty(N, np.int32)
    slot_of = np.empty(N, np.int32)
    for n in order:
        while True:
            esum, cnt, w = heapq.heappop(heap)
            if cnt < P:
                break
        win_of[n] = w
        slot_of[n] = cnt
        heapq.heappush(heap, (esum + int(deg[n]), cnt + 1, w))
    core_of = win_of // NW
    wl_of = win_of % NW
    lrow = wl_of * P + slot_of              # [0, NSLOT) within core

    # ---- edges grouped by dst window, sorted by src
    ewin = win_of[dst]
    ord_e = np.lexsort((src, ewin))
    wcnt = np.bincount(ewin, minlength=WTOT)
    CPW = int(np.ceil(wcnt.max() / P))
    EPW = CPW * P
    starts = np.zeros(WTOT + 1, np.int64)
    np.cumsum(wcnt, out=starts[1:])

    bf = ml_dtypes.bfloat16
    iw1 = np.asarray(inputs["iw1"], np.float32)
    w1 = np.asarray(inputs["w1"], np.float32)
    rw1 = np.asarray(inputs["rw1"], np.float32)
    b1 = np.asarray(inputs["b1"], np.float32)
    ew1 = np.asarray(inputs["ew1"], np.float32)
    eb1 = np.asarray(inputs["eb1"], np.float32)
    iw2 = np.asarray(inputs["iw2"], np.float32)
    w2 = np.asarray(inputs["w2"], np.float32)
    rw2 = np.asarray(inputs["rw2"], np.float32)
    b2 = np.asarray(inputs["b2"], np.float32)
    ew2 = np.asarray(inputs["ew2"], np.float32)
    eb2 = np.asarray(inputs["eb2"], np.float32)

    has_b1 = bool(np.any(b1[1]))
    has_b2 = bool(np.any(b2[1]))
    key = (CPW, has_b1, has_b2)
    nc = _BUILD_CACHE.get(key)
    if nc is None:
        nc = _build_nc(CPW, has_b1, has_b2)
        _BUILD_CACHE[key] = nc

    iota = np.tile(np.arange(P, dtype=np.float32)[None, :], (P, 1))
    ident = np.eye(P, dtype=np.float32)

    def fw1(mats):
        return np.concatenate(mats, axis=1).astype(bf)

    shared = {
        "iota": iota,
        "ident": ident,
        "wh1": fw1([iw1[k] @ w1[0, k] for k in range(K)]),
        "wg1": fw1([rw1[0, k] @ w1[0, k] for k in range(K)]),
        "wbb1": fw1([np.vstack([ew1 @ w1[0, k],
                                (eb1 @ w1[0, k])[None]])
                     for k in range(K)]),
        "waa1": fw1([np.vstack([ew1, (b1[0, k] @ w1[0, k] + eb1)[None]])
                     for k in range(K)]),
        "wrw1": np.concatenate([rw1[1, k] for k in range(K)], 1),
        "b1r": np.concatenate([b1[1, k] for k in range(K)])[None, :],
        "wh2": fw1([iw2[k] @ w2[0, k] for k in range(K)]),
        "wg2": fw1([rw2[0, k] @ w2[0, k] for k in range(K)]),
        "wbb2": fw1([np.vstack([ew2 @ w2[0, k],
                                (eb2 @ w2[0, k])[None]])
                     for k in range(K)]),
        "waa2": fw1([np.vstack([ew2, (b2[0, k] @ w2[0, k] + eb2)[None]])
                     for k in range(K)]),
        "wrw2": np.concatenate([rw2[1, k] for k in range(K)], 1),
        "b2r": np.concatenate([b2[1, k] for k in range(K)])[None, :],
        "g1": np.tile(np.asarray(inputs["gn1_g"], np.float32)[None, :],
                      (P, 1)),
        "bt1": np.tile(np.asarray(inputs["gn1_b"], np.float32)[None, :],
                       (P, 1)),
        "g2": np.tile(np.asarray(inputs["gn2_g"], np.float32)[None, :],
                      (P, 1)),
        "bt2": np.tile(np.asarray(inputs["gn2_b"], np.float32)[None, :],
                       (P, 1)),
    }

    # table row: half-major, rank-major within half, 8 pad rows/half
    H_of = (wl_of >= HW_).astype(np.int64)
    row_of = np.where(
        H_of == 0,
        core_of * HSLAB + wl_of * P + slot_of,
        8 * HSLAB + core_of * HSLAB2 + (wl_of - HW_) * P + slot_of)
    zero_row = HROWS                        # half 0, rank 0 pad row

    # x table (dis*x), full, half-major layout; cols 64:128 zero
    xbtab = np.zeros((8 * NSLAB, P), np.float32)
    xbtab[row_of, :F_IN] = dis[:, None] * x
    shared["xb"] = xbtab.astype(bf)

    in_maps = []
    for c in range(NC):
        idx_all = np.empty((NW, EPW), np.int64)
        slot_all = np.full((NW, EPW), P, np.float32)   # pad slot = 128
        eam_all = np.zeros((NW, EPW, 17), np.float32)
        for wl in range(NW):
            w = c * NW + wl
            es = ord_e[starts[w]:starts[w + 1]]
            ne = len(es)
            idx_all[wl, :] = zero_row
            if ne:
                sr = src[es]
                idx_all[wl, :ne] = row_of[sr]
                slot_all[wl, :ne] = slot_of[dst[es]]
                eam_all[wl, :ne, :E_DIM] = ea[es] * dis[sr][:, None]
                eam_all[wl, :ne, E_DIM] = dis[sr]

        idx_packed = np.concatenate(
            [_pack_idxs(idx_all[wl]) for wl in range(NW)], axis=1)
        slot_a = slot_all.reshape(NW, CPW, P).transpose(2, 0, 1).copy()
        eam_a = (eam_all.reshape(NW, CPW, P, 17)
                 .transpose(2, 0, 1, 3).copy())

        cmask = core_of == c
        Xq = np.zeros((NSLOT, F_IN), np.float32)
        Xq[lrow[cmask]] = x[cmask]
        dcol = np.zeros((P, NW), np.float32)
        dcol[slot_of[cmask], wl_of[cmask]] = dis[cmask]

        in_maps.append(dict(shared,
                            xT=np.ascontiguousarray(Xq.T),
                            eam=eam_a.astype(bf), slot=slot_a,
                            idx=idx_packed, dcol=dcol,
                            dcol2=dcol * dcol))

    from concourse.bass_utils import run_bass_kernel_spmd
    res = run_bass_kernel_spmd(nc, in_maps, core_ids=list(range(8)))
    kernel._last_results = res

    full = np.zeros((N, OUT), np.float32)
    for c in range(NC):
        r = res.results[c]["out"]
        cmask = core_of == c
        full[cmask] = r[lrow[cmask]]
    return full
